# revision 12
# baseline (speedup 1.0000x reference)
"""Trainium2 Bass kernel for nn_MultiHeadAttention (B=2, S=2048, D=1024, H=16, causal).

Sharding across 8 NeuronCores (single SPMD program):
  - Core c owns batch b=c//4 and two 256-token query chunks {p, 7-p} (p=c%4);
    the pairing balances causal attention work.
  - Everything on-chip is bf16 (PSUM accumulation stays fp32): halves HBM +
    collective bytes vs fp32 and enables fast weight loads (FWL) on the PE.
  - Phase 1: project K^T, V (with softmax scale folded into Wk/bk on the
    host), publish both with ONE AllGather (replica groups [[0-3],[4-7]]);
    Q projection + Wo load overlap the collective.  All biases are applied
    with rank-1 matmuls into PSUM (no scalar-engine bias pass).
  - Phase 2: K^T and V for the whole batch live in SBUF.  Heads are
    processed in pairs (feature block = 128 partitions); per key block the
    two heads' score matmuls are row-tiled (partitions 0:64 / 64:128) into
    one 2-bank PSUM tile so they run concurrently, one wide exp covers both,
    causal masking is a single multiplicative bf16 DVE op on the (host
    per-core) staircase, and ctx accumulates in PSUM across ALL 16 key
    blocks (65th stationary column = softmax denominator).  The softmax
    reciprocal is exp(-ln(d)) on the scalar engine (DVE reciprocal is
    8 cyc/elem and was 67us in the fp32 baseline).
  - Phase 3: output projection for the core's own tokens (row-parallel over
    tokens => no reduction); host re-assembles the full output.
"""
import numpy as np
import ml_dtypes

import concourse.bass as bass
import concourse.bacc as bacc
import concourse.mybir as mybir
import concourse.tile as tile
from concourse.bass_utils import run_bass_kernel_spmd
from concourse.tile_rust import add_dep_helper

B, S, D, H, HD = 2, 2048, 1024, 16, 64
NC = 8
P = 128
F32 = mybir.dt.float32
BF = mybir.dt.bfloat16
NPBF = ml_dtypes.bfloat16

KT_N = D * 512           # K^T shard elems  [1024, 512]
V_N = 512 * 16 * 65      # V shard elems    [512 tok, 16 heads, 64+1]
KV_N = KT_N + V_N

TRACE = False        # set True (e.g. from test.py) to capture an NTFF profile
LAST_RESULT = None   # BassKernelResults of the most recent kernel() call

_ACT_PATCHED = False


def _patch_act_tables():
    """Steer Bacc's act-table-load pass to the combined natural_log+exp
    set.  The pass assigns each activation function the FIRST table set
    containing it, so a kernel using both Exp and Ln alternates between
    `exp_and_others` and `natural_log` -- one ~1.3us ACT_TABLE_LOAD per
    transition (17 loads / 22us on the scalar engine for this kernel).
    Hiding Exp/Ln from the earlier sets makes both resolve to the single
    `natural_log_exp_and_others` set (one load total).  List length and
    order are preserved, so the set ids walrus emits stay valid."""
    global _ACT_PATCHED
    if _ACT_PATCHED:
        return
    import concourse.bacc as _bacc
    _orig = _bacc.get_activation_tables

    def _filtered(arch):
        t = _orig(arch)
        fexp = mybir.ActivationFunctionType.Exp
        fln = mybir.ActivationFunctionType.Ln
        out = {}
        for name, fns in t.items():
            if name != "natural_log_exp_and_others" and (
                    fexp in fns or fln in fns):
                fns = fns - {fexp, fln}
            out[name] = fns
        return out

    _bacc.get_activation_tables = _filtered
    _ACT_PATCHED = True


def sel_tokens(p):
    return list(range(256 * p, 256 * p + 256)) + list(
        range(256 * (7 - p), 256 * (7 - p) + 256)
    )


def _kblk(j):
    """Original 128-token key block j -> (rank-in-group, column offset)."""
    q = j // 2
    rr = q if q <= 3 else 7 - q
    off = (0 if q <= 3 else 256) + 128 * (j % 2)
    return rr, off


def _emit(causal: bool, repeat: int = 1):
    nc = bacc.Bacc(trn_type="TRN2", num_devices=NC)
    fexp = mybir.ActivationFunctionType.Exp
    fln = mybir.ActivationFunctionType.Ln
    _patch_act_tables()

    xT = nc.dram_tensor("xT", [D, 512], BF, kind="ExternalInput")
    wqT = nc.dram_tensor("wqT", [D, D], BF, kind="ExternalInput")
    wkT = nc.dram_tensor("wkT", [D, D], BF, kind="ExternalInput")
    wvT = nc.dram_tensor("wvT", [D, D], BF, kind="ExternalInput")
    woT = nc.dram_tensor("woT", [D, D], BF, kind="ExternalInput")
    bq_d = nc.dram_tensor("bq", [1, D], BF, kind="ExternalInput")
    bk_d = nc.dram_tensor("bk", [1, D], BF, kind="ExternalInput")
    bv_d = nc.dram_tensor("bv", [1, D], BF, kind="ExternalInput")
    bo_d = nc.dram_tensor("bo", [1, D], BF, kind="ExternalInput")
    if causal:
        cmb_d = nc.dram_tensor("cmb", [P, 16, 2, 256], BF, kind="ExternalInput")
    outT = nc.dram_tensor("outT", [D, 512], F32, kind="ExternalOutput")

    kt_loc = nc.dram_tensor("kt_loc", [KT_N], BF)
    v_loc = nc.dram_tensor("v_loc", [V_N], BF)
    kt_all = nc.dram_tensor("kt_all", [4, KT_N], BF)
    v_all = nc.dram_tensor("v_all", [4, V_N], BF)

    with tile.TileContext(nc) as tc, \
         tc.tile_pool(name="const", bufs=1) as const, \
         tc.tile_pool(name="w", bufs=2) as wpool, \
         tc.tile_pool(name="big", bufs=1) as big, \
         tc.tile_pool(name="kv", bufs=1) as kvp, \
         tc.tile_pool(name="io", bufs=3) as io, \
         tc.tile_pool(name="vio", bufs=2) as vio, \
         tc.tile_pool(name="oio", bufs=2) as oio, \
         tc.tile_pool(name="ex", bufs=6) as ex, \
         tc.tile_pool(name="sm", bufs=2) as sm, \
         tc.tile_pool(name="ps_sc", bufs=2, space="PSUM") as ps_sc, \
         tc.tile_pool(name="ps_ctx", bufs=2, space="PSUM") as ps_ctx, \
         tc.tile_pool(name="ps_w", bufs=2, space="PSUM") as ps_w:

        # ---------- constants ----------
        ones = const.tile([P, 512], BF)
        nc.gpsimd.memset(ones[:], 1.0)
        bq_sb = const.tile([1, D], BF)
        nc.sync.dma_start(bq_sb[:], bq_d[:])
        bk_sb = const.tile([1, D], BF)
        nc.sync.dma_start(bk_sb[:], bk_d[:])
        bv_sb = const.tile([1, D], BF)
        nc.sync.dma_start(bv_sb[:], bv_d[:])
        bo_sb = const.tile([1, D], BF)
        nc.sync.dma_start(bo_sb[:], bo_d[:])
        if causal:
            cmb_sb = big.tile([P, 16, 2, 256], BF)
            nc.sync.dma_start(cmb_sb[:], cmb_d[:])

        rg = [[0, 1, 2, 3], [4, 5, 6, 7]]
        kt_ap = kt_loc[:].rearrange("(o p t) -> p o t", o=8, p=P, t=512)
        v_ap = v_loc[:].rearrange("(a p h c) -> p a h c",
                                  a=4, p=P, h=16, c=65)

        for _rep in range(repeat):
            # ---------- phase 1: projections for this core's 512 tokens ----
            xt_sb = big.tile([P, 8, 512], BF)
            xr = xT.rearrange("(o p) t -> p o t", p=P)
            for _kt in range(8):
                nc.sync.dma_start(xt_sb[:, _kt, :], xr[:, _kt, :])
            qt_sb = big.tile([P, 8, 512], BF)

            def load_w(w_dram):
                w_sb = wpool.tile([P, 8, D], BF, tag="w")
                wr = w_dram.rearrange("(o p) t -> p o t", p=P)
                for _kt in range(8):
                    nc.sync.dma_start(w_sb[:, _kt, :], wr[:, _kt, :])
                return w_sb

            def proj_T(w_sb, bias_sb, sink):
                # out[feat, tok]: per-partition bias via rank-1 matmul
                for dt in range(8):
                    pt = ps_w.tile([P, 512], F32, tag="psw")
                    for kt in range(8):
                        nc.tensor.matmul(
                            pt[:], w_sb[:, kt, 128 * dt:128 * dt + 128],
                            xt_sb[:, kt, :], start=(kt == 0), stop=False)
                    nc.tensor.matmul(
                        pt[:], bias_sb[0:1, 128 * dt:128 * dt + 128],
                        ones[0:1, 0:512], start=False, stop=True)
                    sink(dt, pt)

            # K^T -> kv_loc
            wk_sb = load_w(wkT)

            def k_sink(dt, pt):
                t = io.tile([P, 512], BF, tag="io")
                nc.vector.tensor_copy(t[:], pt[:])
                nc.sync.dma_start(kt_ap[:, dt, :], t[:])
            proj_T(wk_sb, bk_sb, k_sink)
            # AllGather K^T right away; V projection + AG-V overlap it
            cc_k = nc.gpsimd.collective_compute(
                "AllGather", mybir.AluOpType.bypass, replica_groups=rg,
                ins=[kt_loc[:]], outs=[kt_all[:]])

            # V -> v_loc ([tok, head, 64] + ones column)
            wv_sb = load_w(wvT)
            for st in range(4):
                vt = vio.tile([P, 16, 65], BF, tag="vio")
                for hf in range(2):
                    pt = ps_w.tile([P, 512], F32, tag="psw")
                    for kt in range(8):
                        nc.tensor.matmul(
                            pt[:], xt_sb[:, kt, 128 * st:128 * st + 128],
                            wv_sb[:, kt, 512 * hf:512 * hf + 512],
                            start=(kt == 0), stop=False)
                    nc.tensor.matmul(
                        pt[:], ones[0:1, 0:P],
                        bv_sb[0:1, 512 * hf:512 * hf + 512],
                        start=False, stop=True)
                    nc.vector.tensor_copy(
                        vt[:, 8 * hf:8 * hf + 8, 0:64],
                        pt[:].rearrange("p (h d) -> p h d", h=8))
                nc.vector.tensor_copy(vt[:, :, 64:65], ones[:, 0:16, None])
                nc.sync.dma_start(v_ap[:, st, :, :], vt[:])

            cc_v = nc.gpsimd.collective_compute(
                "AllGather", mybir.AluOpType.bypass, replica_groups=rg,
                ins=[v_loc[:]], outs=[v_all[:]])

            # stage gathered K^T (overlaps AG-V), then Q projection
            kt_sb = kvp.tile([P, 4, 8, 512], BF)
            v_sb = kvp.tile([P, 16, 16, 65], BF)
            for r in range(4):
                src = kt_all[r, :].rearrange("(o p t) -> p o t",
                                             o=8, p=P, t=512)
                dk = nc.sync.dma_start(kt_sb[:, r, :, :], src[:, :, :])
                add_dep_helper(dk.ins, cc_k.ins, reason="read after AG-K")

            # Q (stays in SBUF, bf16) -- overlaps the collectives
            wq_sb = load_w(wqT)

            def q_sink(dt, pt):
                nc.vector.tensor_copy(qt_sb[:, dt, :], pt[:])
            proj_T(wq_sb, bq_sb, q_sink)
            wo_sb = load_w(woT)

            for r in range(4):
                vsrc = v_all[r, :].rearrange(
                    "(a p h c) -> p a h c", a=4, p=P, h=16, c=65)
                for a in range(4):
                    dv = nc.sync.dma_start(v_sb[:, 4 * r + a, :, :],
                                           vsrc[:, a, :, :])
                    add_dep_helper(dv.ins, cc_v.ins, reason="read after AG-V")

            # ---------- phase 2: attention, head pairs ----------
            ctx_sb = big.tile([P, 8, 512], BF)
            for pair in range(8):
                h0, h1 = 2 * pair, 2 * pair + 1
                ctx0 = ps_ctx.tile([P, 512], F32, tag="ctx")
                ctx1 = ps_ctx.tile([P, 512], F32, tag="ctx")
                for j in range(16):
                    wid = 512 if (not causal or j < 8) else 256
                    qoff = 0 if (not causal or j < 8) else 256
                    rr, off = _kblk(j)
                    sc = ps_sc.tile([P, 1024], F32, tag="sc")
                    # two heads row-tiled: run concurrently on the PE
                    nc.tensor.matmul(
                        sc[:, 0:wid],
                        kt_sb[0:64, rr, pair, off:off + 128],
                        qt_sb[0:64, pair, qoff:qoff + wid],
                        start=True, stop=True)
                    nc.tensor.matmul(
                        sc[:, 512:512 + wid],
                        kt_sb[64:128, rr, pair, off:off + 128],
                        qt_sb[64:128, pair, qoff:qoff + wid],
                        start=True, stop=True)
                    et = ex.tile([P, 2, 512], BF, tag="exp")
                    if wid == 512:
                        nc.scalar.activation(et[:, :, :], sc[:, :], fexp)
                    else:
                        nc.scalar.activation(
                            et[:, :, 0:256],
                            sc[:].rearrange("p (s n) -> p s n", s=2)
                            [:, :, 0:256], fexp)
                    if causal:
                        nc.vector.tensor_tensor(
                            et[:, :, 0:256], et[:, :, 0:256],
                            cmb_sb[:, j, :, :], mybir.AluOpType.mult)
                    vj = 4 * rr + off // 128   # v_sb slot (rank-major order)
                    nc.tensor.matmul(
                        ctx0[0:65, qoff:qoff + wid],
                        v_sb[:, vj, h0, 0:65], et[:, 0, 0:wid],
                        start=(j == 0), stop=(j == 15))
                    nc.tensor.matmul(
                        ctx1[0:65, qoff:qoff + wid],
                        v_sb[:, vj, h1, 0:65], et[:, 1, 0:wid],
                        start=(j == 0), stop=(j == 15))
                # normalize: recip = exp(-ln(denominator)); even head lands
                # on partitions 0:64 of ctx_sb, odd head is moved to 64:128
                # with a small SBUF->SBUF DMA (cross-partition).
                for hi, ctxp in ((0, ctx0), (1, ctx1)):
                    lnd = sm.tile([1, 512], F32, tag="lnd")
                    nc.scalar.activation(lnd[:], ctxp[64:65, 0:512], fln)
                    rcp = sm.tile([1, 512], BF, tag="rcp")
                    nc.scalar.activation(rcp[:], lnd[:], fexp, scale=-1.0)
                    rep_ps = ps_w.tile([P, 512], F32, tag="psw")
                    nc.tensor.matmul(rep_ps[0:64, :], ones[0:1, 0:64],
                                     rcp[0:1, :], start=True, stop=True)
                    rep_sb = sm.tile([64, 512], F32, tag="rep")
                    nc.vector.tensor_copy(rep_sb[:], rep_ps[0:64, :])
                    if hi == 0:
                        nc.vector.tensor_tensor(
                            ctx_sb[0:64, pair, :], ctxp[0:64, :],
                            rep_sb[:], mybir.AluOpType.mult)
                    else:
                        ctmp = sm.tile([64, 512], BF, tag="ctmp")
                        nc.vector.tensor_tensor(
                            ctmp[:], ctxp[0:64, :], rep_sb[:],
                            mybir.AluOpType.mult)
                        nc.sync.dma_start(ctx_sb[64:128, pair, :], ctmp[:])

            # ---------- phase 3: output projection ----------
            for m in range(8):
                pt = ps_w.tile([P, 512], F32, tag="psw")
                for kt in range(8):
                    nc.tensor.matmul(
                        pt[:], wo_sb[:, kt, 128 * m:128 * m + 128],
                        ctx_sb[:, kt, :], start=(kt == 0), stop=False)
                nc.tensor.matmul(
                    pt[:], bo_sb[0:1, 128 * m:128 * m + 128],
                    ones[0:1, 0:512], start=False, stop=True)
                t = oio.tile([P, 512], F32, tag="oio")
                nc.vector.tensor_copy(t[:], pt[:])
                nc.sync.dma_start(
                    outT.rearrange("(o p) t -> p o t", p=P)[:, m, :], t[:])

    nc.compile()
    return nc


_CACHE = {}


def _get_nc(causal: bool, repeat: int = 1):
    key = (causal, repeat)
    if key not in _CACHE:
        _CACHE[key] = _emit(causal, repeat)
    return _CACHE[key]


def _mask01(p):
    """Per-core multiplicative mask [128, 16, 2, 256] for the causal
    staircase (same mask for both heads of a pair, hence the dim of 2)."""
    k = np.arange(128)[:, None]
    c = np.arange(256)[None, :]
    m1 = (c - k >= 0).astype(np.float32)
    m2 = (c - 128 - k >= 0).astype(np.float32)
    cmb = np.ones((128, 16, 256), dtype=np.float32)
    # j<8: masks the LOW chunk (cols 0:256 of the 512-wide tile)
    for j in range(8):
        if j == 2 * p:
            cmb[:, j, :] = m1
        elif j == 2 * p + 1:
            cmb[:, j, :] = m2
        elif j > 2 * p + 1:
            cmb[:, j, :] = 0.0
    # j>=8: masks the HIGH chunk (the only 256 cols computed)
    for j in range(8, 16):
        if j == 14 - 2 * p:
            cmb[:, j, :] = m1
        elif j == 15 - 2 * p:
            cmb[:, j, :] = m2
        elif j > 15 - 2 * p:
            cmb[:, j, :] = 0.0
    return np.ascontiguousarray(
        np.broadcast_to(cmb[:, :, None, :], (128, 16, 2, 256))
    ).astype(NPBF)


def kernel(**inputs):
    x = np.asarray(inputs["x"], dtype=np.float32)
    Wq = np.asarray(inputs["Wq"], dtype=np.float32)
    bq = np.asarray(inputs["bq"], dtype=np.float32)
    Wk = np.asarray(inputs["Wk"], dtype=np.float32)
    bk = np.asarray(inputs["bk"], dtype=np.float32)
    Wv = np.asarray(inputs["Wv"], dtype=np.float32)
    bv = np.asarray(inputs["bv"], dtype=np.float32)
    Wo = np.asarray(inputs["Wo"], dtype=np.float32)
    bo = np.asarray(inputs["bo"], dtype=np.float32)
    causal = bool(int(np.asarray(inputs["enable_causal"])))

    scale = np.float32(1.0 / np.sqrt(HD))
    wqT = np.ascontiguousarray(Wq.T).astype(NPBF)
    wkT = np.ascontiguousarray((Wk * scale).T).astype(NPBF)
    wvT = np.ascontiguousarray(Wv.T).astype(NPBF)
    woT = np.ascontiguousarray(Wo.T).astype(NPBF)
    bqr = bq.reshape(1, D).astype(NPBF)
    bkr = (bk * scale).reshape(1, D).astype(NPBF)
    bvr = bv.reshape(1, D).astype(NPBF)
    bor = bo.reshape(1, D).astype(NPBF)

    nc = _get_nc(causal)
    in_maps = []
    for c in range(NC):
        b, p = divmod(c, 4)
        sel = sel_tokens(p)
        xTc = np.ascontiguousarray(x[b][sel, :].T).astype(NPBF)
        m = {"xT": xTc, "wqT": wqT, "wkT": wkT, "wvT": wvT, "woT": woT,
             "bq": bqr, "bk": bkr, "bv": bvr, "bo": bor}
        if causal:
            m["cmb"] = _mask01(p)
        in_maps.append(m)

    global LAST_RESULT
    res = run_bass_kernel_spmd(nc, in_maps, list(range(NC)), trace=TRACE)
    LAST_RESULT = res
    out = np.empty((B, S, D), dtype=np.float32)
    for c in range(NC):
        b, p = divmod(c, 4)
        sel = sel_tokens(p)
        out[b, sel, :] = np.asarray(res.results[c]["outT"], dtype=np.float32).T
    return out


# revision 21
# speedup vs baseline: 1.0617x; 1.0617x over previous
"""Trainium2 Bass kernel for nn_MultiHeadAttention (B=2, S=2048, D=1024, H=16, causal).

Sharding across 8 NeuronCores (single SPMD program):
  - Core c owns batch b=c//4 and two 256-token query chunks {p, 7-p} (p=c%4);
    the pairing balances causal attention work.
  - Everything on-chip is bf16 (PSUM accumulation stays fp32): halves HBM +
    collective bytes vs fp32 and enables fast weight loads (FWL) on the PE.
  - Phase 1: project K^T, V (with softmax scale folded into Wk/bk on the
    host), publish both with ONE AllGather (replica groups [[0-3],[4-7]]);
    Q projection + Wo load overlap the collective.  All biases are applied
    with rank-1 matmuls into PSUM (no scalar-engine bias pass).
  - Phase 2: K^T and V for the whole batch live in SBUF.  Heads are
    processed in pairs (feature block = 128 partitions); per key block the
    two heads' score matmuls are row-tiled (partitions 0:64 / 64:128) into
    one 2-bank PSUM tile so they run concurrently, one wide exp covers both,
    causal masking is a single multiplicative bf16 DVE op on the (host
    per-core) staircase, and ctx accumulates in PSUM across ALL 16 key
    blocks (65th stationary column = softmax denominator).  The softmax
    reciprocal is exp(-ln(d)) on the scalar engine (DVE reciprocal is
    8 cyc/elem and was 67us in the fp32 baseline).
  - Phase 3: output projection for the core's own tokens (row-parallel over
    tokens => no reduction); host re-assembles the full output.
"""
import numpy as np
import ml_dtypes

import concourse.bass as bass
import concourse.bacc as bacc
import concourse.mybir as mybir
import concourse.tile as tile
from concourse.bass_utils import run_bass_kernel_spmd
from concourse.tile_rust import add_dep_helper

B, S, D, H, HD = 2, 2048, 1024, 16, 64
NC = 8
P = 128
F32 = mybir.dt.float32
BF = mybir.dt.bfloat16
NPBF = ml_dtypes.bfloat16

KT_N = D * 512           # K^T shard elems  [1024, 512]
V_N = 512 * 16 * 65      # V shard elems    [512 tok, 16 heads, 64+1]
KV_N = KT_N + V_N

TRACE = False        # set True (e.g. from test.py) to capture an NTFF profile
LAST_RESULT = None   # BassKernelResults of the most recent kernel() call

_ACT_PATCHED = False


def _patch_act_tables():
    """Steer Bacc's act-table-load pass to the combined natural_log+exp
    set.  The pass assigns each activation function the FIRST table set
    containing it, so a kernel using both Exp and Ln alternates between
    `exp_and_others` and `natural_log` -- one ~1.3us ACT_TABLE_LOAD per
    transition (17 loads / 22us on the scalar engine for this kernel).
    Hiding Exp/Ln from the earlier sets makes both resolve to the single
    `natural_log_exp_and_others` set (one load total).  List length and
    order are preserved, so the set ids walrus emits stay valid."""
    global _ACT_PATCHED
    if _ACT_PATCHED:
        return
    import concourse.bacc as _bacc
    _orig = _bacc.get_activation_tables

    def _filtered(arch):
        t = _orig(arch)
        fexp = mybir.ActivationFunctionType.Exp
        fln = mybir.ActivationFunctionType.Ln
        out = {}
        for name, fns in t.items():
            if name != "natural_log_exp_and_others" and (
                    fexp in fns or fln in fns):
                fns = fns - {fexp, fln}
            out[name] = fns
        return out

    _bacc.get_activation_tables = _filtered
    _ACT_PATCHED = True


def sel_tokens(p):
    return list(range(256 * p, 256 * p + 256)) + list(
        range(256 * (7 - p), 256 * (7 - p) + 256)
    )


def _kblk(j):
    """Original 128-token key block j -> (rank-in-group, column offset)."""
    q = j // 2
    rr = q if q <= 3 else 7 - q
    off = (0 if q <= 3 else 256) + 128 * (j % 2)
    return rr, off


def _emit(causal: bool, repeat: int = 1):
    nc = bacc.Bacc(trn_type="TRN2", num_devices=NC)
    fexp = mybir.ActivationFunctionType.Exp
    fln = mybir.ActivationFunctionType.Ln
    _patch_act_tables()

    xT = nc.dram_tensor("xT", [D, 512], BF, kind="ExternalInput")
    wqT = nc.dram_tensor("wqT", [D, D], BF, kind="ExternalInput")
    wkT = nc.dram_tensor("wkT", [D, D], BF, kind="ExternalInput")
    wvT = nc.dram_tensor("wvT", [D, D], BF, kind="ExternalInput")
    woT = nc.dram_tensor("woT", [D, D], BF, kind="ExternalInput")
    bq_d = nc.dram_tensor("bq", [1, D], BF, kind="ExternalInput")
    bk_d = nc.dram_tensor("bk", [1, D], BF, kind="ExternalInput")
    bv_d = nc.dram_tensor("bv", [1, D], BF, kind="ExternalInput")
    bo_d = nc.dram_tensor("bo", [1, D], BF, kind="ExternalInput")
    if causal:
        cmb_d = nc.dram_tensor("cmb", [P, 16, 2, 256], BF, kind="ExternalInput")
    outT = nc.dram_tensor("outT", [D, 512], F32, kind="ExternalOutput")

    kv_loc = nc.dram_tensor("kv_loc", [KV_N], BF)
    kv_all = nc.dram_tensor("kv_all", [4, KV_N], BF)

    with tile.TileContext(nc) as tc, \
         tc.tile_pool(name="const", bufs=1) as const, \
         tc.tile_pool(name="w", bufs=2) as wpool, \
         tc.tile_pool(name="big", bufs=1) as big, \
         tc.tile_pool(name="kv", bufs=1) as kvp, \
         tc.tile_pool(name="io", bufs=3) as io, \
         tc.tile_pool(name="vio", bufs=2) as vio, \
         tc.tile_pool(name="oio", bufs=2) as oio, \
         tc.tile_pool(name="ex", bufs=4) as ex, \
         tc.tile_pool(name="sm", bufs=2) as sm, \
         tc.tile_pool(name="ps_sc", bufs=2, space="PSUM") as ps_sc, \
         tc.tile_pool(name="ps_ctx", bufs=2, space="PSUM") as ps_ctx, \
         tc.tile_pool(name="ps_w", bufs=2, space="PSUM") as ps_w:

        # ---------- constants ----------
        ones = const.tile([P, 512], BF)
        nc.gpsimd.memset(ones[:], 1.0)
        bq_sb = const.tile([1, D], BF)
        nc.sync.dma_start(bq_sb[:], bq_d[:])
        bk_sb = const.tile([1, D], BF)
        nc.sync.dma_start(bk_sb[:], bk_d[:])
        bv_sb = const.tile([1, D], BF)
        nc.sync.dma_start(bv_sb[:], bv_d[:])
        bo_sb = const.tile([1, D], BF)
        nc.sync.dma_start(bo_sb[:], bo_d[:])
        if causal:
            cmb_sb = big.tile([P, 16, 2, 256], BF)

        rg = [[0, 1, 2, 3], [4, 5, 6, 7]]
        kt_ap = kv_loc[0:KT_N].rearrange("(o p t) -> p o t", o=8, p=P, t=512)
        v_ap = kv_loc[KT_N:KV_N].rearrange("(a p h c) -> p a h c",
                                           a=4, p=P, h=16, c=65)

        for _rep in range(repeat):
            # ---------- phase 1: projections for this core's 512 tokens ----
            xt_sb = big.tile([P, 8, 512], BF)
            xr = xT.rearrange("(o p) t -> p o t", p=P)
            for _kt in range(8):
                nc.sync.dma_start(xt_sb[:, _kt, :], xr[:, _kt, :])
            qt_sb = big.tile([P, 8, 512], BF)

            def load_w(w_dram):
                w_sb = wpool.tile([P, 8, D], BF, tag="w")
                wr = w_dram.rearrange("(o p) t -> p o t", p=P)
                for _kt in range(8):
                    nc.sync.dma_start(w_sb[:, _kt, :], wr[:, _kt, :])
                return w_sb

            def proj_T(w_sb, bias_sb, sink):
                # out[feat, tok]: per-partition bias via rank-1 matmul
                for dt in range(8):
                    pt = ps_w.tile([P, 512], F32, tag="psw")
                    for kt in range(8):
                        nc.tensor.matmul(
                            pt[:], w_sb[:, kt, 128 * dt:128 * dt + 128],
                            xt_sb[:, kt, :], start=(kt == 0), stop=False)
                    nc.tensor.matmul(
                        pt[:], bias_sb[0:1, 128 * dt:128 * dt + 128],
                        ones[0:1, 0:512], start=False, stop=True)
                    sink(dt, pt)

            # K^T -> kv_loc
            wk_sb = load_w(wkT)

            def k_sink(dt, pt):
                t = io.tile([P, 512], BF, tag="io")
                nc.vector.tensor_copy(t[:], pt[:])
                nc.sync.dma_start(kt_ap[:, dt, :], t[:])
            proj_T(wk_sb, bk_sb, k_sink)

            # V -> kv_loc ([tok, head, 64] + ones column)
            wv_sb = load_w(wvT)
            for st in range(4):
                vt = vio.tile([P, 16, 65], BF, tag="vio")
                for hf in range(2):
                    pt = ps_w.tile([P, 512], F32, tag="psw")
                    for kt in range(8):
                        nc.tensor.matmul(
                            pt[:], xt_sb[:, kt, 128 * st:128 * st + 128],
                            wv_sb[:, kt, 512 * hf:512 * hf + 512],
                            start=(kt == 0), stop=False)
                    nc.tensor.matmul(
                        pt[:], ones[0:1, 0:P],
                        bv_sb[0:1, 512 * hf:512 * hf + 512],
                        start=False, stop=True)
                    nc.vector.tensor_copy(
                        vt[:, 8 * hf:8 * hf + 8, 0:64],
                        pt[:].rearrange("p (h d) -> p h d", h=8))
                nc.vector.tensor_copy(vt[:, :, 64:65], ones[:, 0:16, None])
                nc.sync.dma_start(v_ap[:, st, :, :], vt[:])

            # ONE AllGather for K^T + V
            cc = nc.gpsimd.collective_compute(
                "AllGather", mybir.AluOpType.bypass, replica_groups=rg,
                ins=[kv_loc[:]], outs=[kv_all[:]])

            # mask load deferred to here: first needed by attention, and at
            # t=0 its 2MB competed with the x/weight DMAs feeding the PE
            if causal:
                nc.sync.dma_start(cmb_sb[:], cmb_d[:])

            # Q (stays in SBUF, bf16) -- overlaps the collective
            wq_sb = load_w(wqT)

            def q_sink(dt, pt):
                nc.vector.tensor_copy(qt_sb[:, dt, :], pt[:])
            proj_T(wq_sb, bq_sb, q_sink)
            wo_sb = load_w(woT)

            # stage gathered K^T / V into SBUF
            kt_sb = kvp.tile([P, 4, 8, 512], BF)
            v_sb = kvp.tile([P, 16, 16, 65], BF)
            for r in range(4):
                src = kv_all[r, 0:KT_N].rearrange("(o p t) -> p o t",
                                                  o=8, p=P, t=512)
                dk = nc.sync.dma_start(kt_sb[:, r, :, :], src[:, :, :])
                add_dep_helper(dk.ins, cc.ins, reason="read after AG")
                vsrc = kv_all[r, KT_N:KV_N].rearrange(
                    "(a p h c) -> p a h c", a=4, p=P, h=16, c=65)
                for a in range(4):
                    dv = nc.sync.dma_start(v_sb[:, 4 * r + a, :, :],
                                           vsrc[:, a, :, :])
                    add_dep_helper(dv.ins, cc.ins, reason="read after AG")

            # ---------- phase 2: attention, head pairs ----------
            ctx_sb = big.tile([P, 8, 512], BF)
            for pair in range(8):
                h0, h1 = 2 * pair, 2 * pair + 1
                ctx0 = ps_ctx.tile([P, 512], F32, tag="ctx")
                ctx1 = ps_ctx.tile([P, 512], F32, tag="ctx")
                for j in range(16):
                    wid = 512 if (not causal or j < 8) else 256
                    qoff = 0 if (not causal or j < 8) else 256
                    rr, off = _kblk(j)
                    sc = ps_sc.tile([P, 1024], F32, tag="sc")
                    # two heads row-tiled: run concurrently on the PE
                    nc.tensor.matmul(
                        sc[:, 0:wid],
                        kt_sb[0:64, rr, pair, off:off + 128],
                        qt_sb[0:64, pair, qoff:qoff + wid],
                        start=True, stop=True)
                    nc.tensor.matmul(
                        sc[:, 512:512 + wid],
                        kt_sb[64:128, rr, pair, off:off + 128],
                        qt_sb[64:128, pair, qoff:qoff + wid],
                        start=True, stop=True)
                    et = ex.tile([P, 2, 512], BF, tag="exp")
                    if wid == 512:
                        nc.scalar.activation(et[:, :, :], sc[:, :], fexp)
                    else:
                        nc.scalar.activation(
                            et[:, :, 0:256],
                            sc[:].rearrange("p (s n) -> p s n", s=2)
                            [:, :, 0:256], fexp)
                    if causal:
                        nc.vector.tensor_tensor(
                            et[:, :, 0:256], et[:, :, 0:256],
                            cmb_sb[:, j, :, :], mybir.AluOpType.mult)
                    vj = 4 * rr + off // 128   # v_sb slot (rank-major order)
                    nc.tensor.matmul(
                        ctx0[0:65, qoff:qoff + wid],
                        v_sb[:, vj, h0, 0:65], et[:, 0, 0:wid],
                        start=(j == 0), stop=(j == 15))
                    nc.tensor.matmul(
                        ctx1[0:65, qoff:qoff + wid],
                        v_sb[:, vj, h1, 0:65], et[:, 1, 0:wid],
                        start=(j == 0), stop=(j == 15))
                # normalize: recip = exp(-ln(denominator)); the two heads'
                # chains are interleaved stage-by-stage so the ACT/DVE ops
                # pipeline and the ctx banks free sooner.  Even head lands
                # on partitions 0:64 of ctx_sb, odd head is moved to 64:128
                # with a small SBUF->SBUF DMA (cross-partition).
                lnd0 = sm.tile([1, 512], F32, tag="lnd")
                nc.scalar.activation(lnd0[:], ctx0[64:65, 0:512], fln)
                lnd1 = sm.tile([1, 512], F32, tag="lnd")
                nc.scalar.activation(lnd1[:], ctx1[64:65, 0:512], fln)
                rcp0 = sm.tile([1, 512], BF, tag="rcp")
                nc.scalar.activation(rcp0[:], lnd0[:], fexp, scale=-1.0)
                rcp1 = sm.tile([1, 512], BF, tag="rcp")
                nc.scalar.activation(rcp1[:], lnd1[:], fexp, scale=-1.0)
                rep_ps0 = ps_w.tile([P, 512], F32, tag="psw")
                nc.tensor.matmul(rep_ps0[0:64, :], ones[0:1, 0:64],
                                 rcp0[0:1, :], start=True, stop=True)
                rep_ps1 = ps_w.tile([P, 512], F32, tag="psw")
                nc.tensor.matmul(rep_ps1[0:64, :], ones[0:1, 0:64],
                                 rcp1[0:1, :], start=True, stop=True)
                rep0 = sm.tile([64, 512], F32, tag="rep")
                nc.vector.tensor_copy(rep0[:], rep_ps0[0:64, :])
                rep1 = sm.tile([64, 512], F32, tag="rep")
                nc.vector.tensor_copy(rep1[:], rep_ps1[0:64, :])
                nc.vector.tensor_tensor(
                    ctx_sb[0:64, pair, :], ctx0[0:64, :],
                    rep0[:], mybir.AluOpType.mult)
                ctmp = sm.tile([64, 512], BF, tag="ctmp")
                nc.vector.tensor_tensor(
                    ctmp[:], ctx1[0:64, :], rep1[:],
                    mybir.AluOpType.mult)
                nc.sync.dma_start(ctx_sb[64:128, pair, :], ctmp[:])

            # ---------- phase 3: output projection ----------
            for m in range(8):
                pt = ps_w.tile([P, 512], F32, tag="psw")
                for kt in range(8):
                    nc.tensor.matmul(
                        pt[:], wo_sb[:, kt, 128 * m:128 * m + 128],
                        ctx_sb[:, kt, :], start=(kt == 0), stop=False)
                nc.tensor.matmul(
                    pt[:], bo_sb[0:1, 128 * m:128 * m + 128],
                    ones[0:1, 0:512], start=False, stop=True)
                t = oio.tile([P, 512], F32, tag="oio")
                nc.vector.tensor_copy(t[:], pt[:])
                nc.sync.dma_start(
                    outT.rearrange("(o p) t -> p o t", p=P)[:, m, :], t[:])

    nc.compile()
    return nc


_CACHE = {}


def _get_nc(causal: bool, repeat: int = 1):
    key = (causal, repeat)
    if key not in _CACHE:
        _CACHE[key] = _emit(causal, repeat)
    return _CACHE[key]


def _mask01(p):
    """Per-core multiplicative mask [128, 16, 2, 256] for the causal
    staircase (same mask for both heads of a pair, hence the dim of 2)."""
    k = np.arange(128)[:, None]
    c = np.arange(256)[None, :]
    m1 = (c - k >= 0).astype(np.float32)
    m2 = (c - 128 - k >= 0).astype(np.float32)
    cmb = np.ones((128, 16, 256), dtype=np.float32)
    # j<8: masks the LOW chunk (cols 0:256 of the 512-wide tile)
    for j in range(8):
        if j == 2 * p:
            cmb[:, j, :] = m1
        elif j == 2 * p + 1:
            cmb[:, j, :] = m2
        elif j > 2 * p + 1:
            cmb[:, j, :] = 0.0
    # j>=8: masks the HIGH chunk (the only 256 cols computed)
    for j in range(8, 16):
        if j == 14 - 2 * p:
            cmb[:, j, :] = m1
        elif j == 15 - 2 * p:
            cmb[:, j, :] = m2
        elif j > 15 - 2 * p:
            cmb[:, j, :] = 0.0
    return np.ascontiguousarray(
        np.broadcast_to(cmb[:, :, None, :], (128, 16, 2, 256))
    ).astype(NPBF)


def kernel(**inputs):
    x = np.asarray(inputs["x"], dtype=np.float32)
    Wq = np.asarray(inputs["Wq"], dtype=np.float32)
    bq = np.asarray(inputs["bq"], dtype=np.float32)
    Wk = np.asarray(inputs["Wk"], dtype=np.float32)
    bk = np.asarray(inputs["bk"], dtype=np.float32)
    Wv = np.asarray(inputs["Wv"], dtype=np.float32)
    bv = np.asarray(inputs["bv"], dtype=np.float32)
    Wo = np.asarray(inputs["Wo"], dtype=np.float32)
    bo = np.asarray(inputs["bo"], dtype=np.float32)
    causal = bool(int(np.asarray(inputs["enable_causal"])))

    scale = np.float32(1.0 / np.sqrt(HD))
    wqT = np.ascontiguousarray(Wq.T).astype(NPBF)
    wkT = np.ascontiguousarray((Wk * scale).T).astype(NPBF)
    wvT = np.ascontiguousarray(Wv.T).astype(NPBF)
    woT = np.ascontiguousarray(Wo.T).astype(NPBF)
    bqr = bq.reshape(1, D).astype(NPBF)
    bkr = (bk * scale).reshape(1, D).astype(NPBF)
    bvr = bv.reshape(1, D).astype(NPBF)
    bor = bo.reshape(1, D).astype(NPBF)

    nc = _get_nc(causal)
    in_maps = []
    for c in range(NC):
        b, p = divmod(c, 4)
        sel = sel_tokens(p)
        xTc = np.ascontiguousarray(x[b][sel, :].T).astype(NPBF)
        m = {"xT": xTc, "wqT": wqT, "wkT": wkT, "wvT": wvT, "woT": woT,
             "bq": bqr, "bk": bkr, "bv": bvr, "bo": bor}
        if causal:
            m["cmb"] = _mask01(p)
        in_maps.append(m)

    global LAST_RESULT
    res = run_bass_kernel_spmd(nc, in_maps, list(range(NC)), trace=TRACE)
    LAST_RESULT = res
    out = np.empty((B, S, D), dtype=np.float32)
    for c in range(NC):
        b, p = divmod(c, 4)
        sel = sel_tokens(p)
        out[b, sel, :] = np.asarray(res.results[c]["outT"], dtype=np.float32).T
    return out
